# revision 2
# baseline (speedup 1.0000x reference)
"""Distributed causal attention head on 8 TRN2 NeuronCores.

Problem: B=4, S=4096, D_in=512, D_out=64 causal attention
  K/V/Q = X @ W; scores = Q@K^T (causal, /sqrt(64)); Z = softmax(scores)@V

Sharding: core c = 2*b + h handles batch b, seq-half h.
q-rows are interleaved at 128-row-block granularity (core h owns global
q-blocks {2j+h}), which makes the causal block schedule IDENTICAL on all
cores (SPMD-safe) and balances FLOPs exactly.  Every core loads the full
(transposed) K/V inputs of its batch and projects them locally.

Device layout (per core):
  inputs (host-pretransposed, bf16): xq [512,2048], xk/xv [512,4096],
    wq/wk/wv [512,64], cmask [8,128,512]
  QpT/KpT [64,*] e-major, Vp [128-blocks, 64+ones] k-major
  scores computed transposed: ST[k,q] = KpT_blk^T-free matmul, exp on ACT
  (scale=1/8 folded in, no max-subtraction: |scores/8| < ~1.5), ET bf16.
  Z^T accumulated in PSUM via AV matmuls with a ones-column in Vp giving
  the softmax denominator for free; normalize via reciprocal + K=1
  broadcast matmul; out [64,2048] f32 (host transposes back).
"""

import numpy as np
import ml_dtypes

import concourse.bass as bass
import concourse.bacc as bacc
import concourse.mybir as mybir
import concourse.tile as tile

B, S, D, E = 4, 4096, 512, 64
PB = 128                      # partition block
NKB = S // PB                 # 32 k-blocks (global)
NLQ = NKB // 2                # 16 local q-blocks per core
NCH = 4                       # q-chunks of 512 per core
CHW = 512                     # q-chunk width
ND = D // PB                  # 4 d-slices
BF16 = mybir.dt.bfloat16
F32 = mybir.dt.float32
NPBF16 = ml_dtypes.bfloat16


def build_nc(group=3):
    """Build the single-core SPMD graph. group = kblocks per exp batch."""
    nc = bacc.Bacc(None)

    xq_d = nc.declare_dram_parameter("xq", [D, S // 2], BF16, isOutput=False)
    xk_d = nc.declare_dram_parameter("xk", [D, S], BF16, isOutput=False)
    xv_d = nc.declare_dram_parameter("xv", [D, S], BF16, isOutput=False)
    wq_d = nc.declare_dram_parameter("wq", [D, E], BF16, isOutput=False)
    wk_d = nc.declare_dram_parameter("wk", [D, E], BF16, isOutput=False)
    wv_d = nc.declare_dram_parameter("wv", [D, E], BF16, isOutput=False)
    cm_d = nc.declare_dram_parameter("cmask", [8, PB, CHW], BF16, isOutput=False)
    out_d = nc.declare_dram_parameter("out", [E, S // 2], F32, isOutput=True)

    with tile.TileContext(nc) as tc:
        with tc.tile_pool(name="persist", bufs=1) as pp:
            # ---- persistent SBUF tiles ----
            wq_sb = [pp.tile([PB, E], BF16, name=f"wq{i}", tag=f"wq{i}") for i in range(ND)]
            wk_sb = [pp.tile([PB, E], BF16, name=f"wk{i}", tag=f"wk{i}") for i in range(ND)]
            wv_sb = [pp.tile([PB, E], BF16, name=f"wv{i}", tag=f"wv{i}") for i in range(ND)]
            xq_sb = [pp.tile([PB, S // 2], BF16, name=f"xq{i}", tag=f"xq{i}") for i in range(ND)]
            xk_sb = [pp.tile([PB, S], BF16, name=f"xk{i}", tag=f"xk{i}") for i in range(ND)]
            xv_sb = [pp.tile([PB, S], BF16, name=f"xv{i}", tag=f"xv{i}") for i in range(ND)]
            mk_sb = [pp.tile([PB, CHW], BF16, name=f"mk{i}", tag=f"mk{i}") for i in range(8)]
            qpT = pp.tile([E, S // 2], BF16, name="qpT", tag="qpT")
            kpT = pp.tile([E, S], BF16, name="kpT", tag="kpT")
            vp = pp.tile([PB, NKB * (E + 1)], BF16, name="vp", tag="vp")
            ones_sb = pp.tile([1, E], F32, name="ones_sb", tag="ones_sb")

            # ---- input DMAs (emission order = rough priority) ----
            for i in range(ND):
                nc.sync.dma_start(out=wq_sb[i][:], in_=wq_d[PB * i:PB * (i + 1), :])
                nc.sync.dma_start(out=wk_sb[i][:], in_=wk_d[PB * i:PB * (i + 1), :])
                nc.sync.dma_start(out=wv_sb[i][:], in_=wv_d[PB * i:PB * (i + 1), :])
            for i in range(ND):
                nc.sync.dma_start(out=xq_sb[i][:], in_=xq_d[PB * i:PB * (i + 1), :])
            for i in range(ND):
                nc.sync.dma_start(out=xk_sb[i][:], in_=xk_d[PB * i:PB * (i + 1), :])
            for i in range(ND):
                nc.sync.dma_start(out=xv_sb[i][:], in_=xv_d[PB * i:PB * (i + 1), :])
            for m in range(8):
                nc.sync.dma_start(out=mk_sb[m][:], in_=cm_d[m])
            nc.vector.memset(ones_sb[:], 1.0)
            nc.vector.memset(vp[:], 1.0)  # pre-fill ones columns

            # ---- projections ----
            with tc.tile_pool(name="proj_ps", bufs=2, space="PSUM") as prp:
                # Q projection: QpT[e, s] chunks of 512
                for c in range(NCH):
                    qp_ps = prp.tile([E, CHW], F32, tag="projps")
                    for d in range(ND):
                        nc.tensor.matmul(
                            qp_ps[:], wq_sb[d][:], xq_sb[d][:, CHW * c:CHW * (c + 1)],
                            start=(d == 0), stop=(d == ND - 1))
                    nc.vector.tensor_copy(qpT[:, CHW * c:CHW * (c + 1)], qp_ps[:])
                # K projection: KpT[e, s] full S
                for c in range(S // CHW):
                    kp_ps = prp.tile([E, CHW], F32, tag="projps")
                    for d in range(ND):
                        nc.tensor.matmul(
                            kp_ps[:], wk_sb[d][:], xk_sb[d][:, CHW * c:CHW * (c + 1)],
                            start=(d == 0), stop=(d == ND - 1))
                    nc.vector.tensor_copy(kpT[:, CHW * c:CHW * (c + 1)], kp_ps[:])
                # V projection: Vp blocks [128, 64] (k on partitions)
                for s in range(NKB):
                    vp_ps = prp.tile([PB, E], F32, tag="projps")
                    for d in range(ND):
                        nc.tensor.matmul(
                            vp_ps[:], xv_sb[d][:, PB * s:PB * (s + 1)], wv_sb[d][:],
                            start=(d == 0), stop=(d == ND - 1))
                    nc.vector.tensor_copy(
                        vp[:, (E + 1) * s:(E + 1) * s + E], vp_ps[:])

            # ---- attention ----
            with tc.tile_pool(name="st_ps", bufs=2, space="PSUM") as stp, \
                 tc.tile_pool(name="zt_ps", bufs=1, space="PSUM") as ztp, \
                 tc.tile_pool(name="rb_ps", bufs=1, space="PSUM") as rbp, \
                 tc.tile_pool(name="work", bufs=3) as wp, \
                 tc.tile_pool(name="osb", bufs=2) as op:
                for c in range(NCH):
                    nkb = 8 * c + 8          # padded kblock count for this chunk
                    zt_ps = ztp.tile([E + 1, CHW], F32, tag="zt")
                    # group kblocks for batched exp
                    groups = []
                    k0 = 0
                    while k0 < nkb:
                        g = min(group, nkb - k0)
                        groups.append(list(range(k0, k0 + g)))
                        k0 += g
                    pend = None  # (et_sb, kbs) awaiting AV
                    for gi, kbs in enumerate(groups):
                        gw = len(kbs) * CHW
                        st_ps = stp.tile([PB, group * CHW], F32, tag="st")
                        for ji, kb in enumerate(kbs):
                            nc.tensor.matmul(
                                st_ps[:, CHW * ji:CHW * (ji + 1)],
                                kpT[:, PB * kb:PB * (kb + 1)],
                                qpT[:, CHW * c:CHW * (c + 1)],
                                start=True, stop=True)
                        # drain previous group's AVs (1-group software pipeline)
                        if pend is not None:
                            p_et, p_kbs = pend
                            for ji, kb in enumerate(p_kbs):
                                nc.tensor.matmul(
                                    zt_ps[:], vp[:, (E + 1) * kb:(E + 1) * (kb + 1)],
                                    p_et[:, CHW * ji:CHW * (ji + 1)],
                                    start=(p_kbs[0] == 0 and ji == 0), stop=False,
                                    skip_group_check=True)
                        et_sb = wp.tile([PB, group * CHW], BF16, tag="et")
                        nc.scalar.activation(
                            et_sb[:, :gw], st_ps[:, :gw],
                            mybir.ActivationFunctionType.Exp, scale=0.125)
                        for ji, kb in enumerate(kbs):
                            m = kb - 8 * c
                            if m >= 0:
                                nc.vector.tensor_mul(
                                    et_sb[:, CHW * ji:CHW * (ji + 1)],
                                    et_sb[:, CHW * ji:CHW * (ji + 1)],
                                    mk_sb[m][:])
                        pend = (et_sb, kbs)
                    p_et, p_kbs = pend
                    for ji, kb in enumerate(p_kbs):
                        nc.tensor.matmul(
                            zt_ps[:], vp[:, (E + 1) * kb:(E + 1) * (kb + 1)],
                            p_et[:, CHW * ji:CHW * (ji + 1)],
                            start=False, stop=(ji == len(p_kbs) - 1),
                            skip_group_check=True)
                    # normalize: row E of zt_ps is the softmax denominator
                    recip_sb = wp.tile([1, CHW], F32, tag="recip")
                    nc.vector.reciprocal(recip_sb[:], zt_ps[E:E + 1, :])
                    rb_ps = rbp.tile([E, CHW], F32, tag="rb")
                    nc.tensor.matmul(rb_ps[:], ones_sb[:], recip_sb[:],
                                     start=True, stop=True)
                    rb_sb = wp.tile([E, CHW], F32, tag="rbsb")
                    nc.vector.tensor_copy(rb_sb[:], rb_ps[:])
                    out_sb = op.tile([E, CHW], F32, tag="outsb")
                    nc.vector.tensor_mul(out_sb[:], zt_ps[:E, :], rb_sb[:])
                    nc.sync.dma_start(out=out_d[:, CHW * c:CHW * (c + 1)],
                                      in_=out_sb[:])
    nc.finalize()
    return nc


def make_core_inputs(key_np, value_np, query_np, Wk, Wv, Wq):
    """Host-side sharding: returns in_maps list of 8 dicts."""
    bf = lambda a: np.ascontiguousarray(a).astype(NPBF16)
    in_maps = []
    for c in range(8):
        b, h = c // 2, c % 2
        qrows = np.concatenate(
            [np.arange(PB * (2 * j + h), PB * (2 * j + h) + PB) for j in range(NLQ)])
        # causal masks: mask m applies to kblock kb = 8c+m of every chunk;
        # section jj (q sub-block) has global q-block g = 8c+2jj+h,
        # class = m - 2jj - h: <0 keep, ==0 triangular, >0 zero.
        cmask = np.zeros((8, PB, CHW), dtype=np.float32)
        ki = np.arange(PB)[:, None]
        qi = np.arange(PB)[None, :]
        tri = (ki <= qi).astype(np.float32)
        for m in range(8):
            for jj in range(4):
                cls = m - 2 * jj - h
                blk = np.ones((PB, PB), np.float32) if cls < 0 else (
                    tri if cls == 0 else np.zeros((PB, PB), np.float32))
                cmask[m][:, PB * jj:PB * (jj + 1)] = blk
        in_maps.append({
            "xq": bf(query_np[b][qrows].T),
            "xk": bf(key_np[b].T),
            "xv": bf(value_np[b].T),
            "wq": bf(Wq), "wk": bf(Wk), "wv": bf(Wv),
            "cmask": bf(cmask),
        })
    return in_maps


def assemble_output(results):
    """results: list of 8 dicts with 'out' [64, 2048] f32 -> Z [B,S,E]."""
    Z = np.zeros((B, S, E), dtype=np.float32)
    for c in range(8):
        b, h = c // 2, c % 2
        o = results[c]["out"]  # [E, 2048]
        for j in range(NLQ):
            g = 2 * j + h
            Z[b, PB * g:PB * (g + 1), :] = o[:, PB * j:PB * (j + 1)].T
    return Z


def kernel(key_inputs, value_inputs, query_inputs, Wk, Wv, Wq):
    from concourse.bass_utils import run_bass_kernel_spmd
    nc = build_nc()
    in_maps = make_core_inputs(np.asarray(key_inputs), np.asarray(value_inputs),
                               np.asarray(query_inputs), np.asarray(Wk),
                               np.asarray(Wv), np.asarray(Wq))
    res = run_bass_kernel_spmd(nc, in_maps, core_ids=list(range(8)))
    return assemble_output(res.results)


# revision 3
# speedup vs baseline: 1.1804x; 1.1804x over previous
"""Distributed causal attention head on 8 TRN2 NeuronCores.

Problem: B=4, S=4096, D_in=512, D_out=64 causal attention
  K/V/Q = X @ W; scores = Q@K^T (causal, /sqrt(64)); Z = softmax(scores)@V

Sharding: core c = 2*b + h handles batch b, seq-half h.
q-rows are interleaved at 128-row-block granularity (core h owns global
q-blocks {2j+h}), which makes the causal block schedule IDENTICAL on all
cores (SPMD-safe) and balances FLOPs exactly.  Every core loads the full
(transposed) K/V inputs of its batch and projects them locally.

Device pipeline (per core, all matmul inputs bf16, psum/softmax f32):
  QpT/KpT projected e-major; KpT2 parity-packed [128, 2048] so score
  matmuls run row-tiled PAIRS (two K=64 matmuls concurrently in the
  128x128 array).  V projected e-major then PE-transposed to k-major
  blocks with an appended ones-column.  Scores computed transposed
  ST[k,q]; exp on ACT in groups of 3 kblocks (scale=1/8 folded in; no
  max-subtraction: |scores/8| < ~1.5).  AV matmuls accumulate Z^T in
  PSUM; the ones-column gives the softmax denominator for free.  Z^T is
  PE-transposed back to q-major, normalized with a per-partition
  reciprocal + tensor_scalar_mul, and DMA'd out q-major [2048, 64].
"""

import numpy as np
import ml_dtypes

import concourse.bass as bass
import concourse.bacc as bacc
import concourse.mybir as mybir
import concourse.tile as tile

B, S, D, E = 4, 4096, 512, 64
PB = 128                      # partition block
NKB = S // PB                 # 32 k-blocks (global)
NLQ = NKB // 2                # 16 local q-blocks per core
NCH = 4                       # q-chunks of 512 per core
CHW = 512                     # q-chunk width
ND = D // PB                  # 4 d-slices
GRP = 3                       # kblocks per exp group
LAG = 2                       # ST->AV software pipeline depth (groups)
BF16 = mybir.dt.bfloat16
F32 = mybir.dt.float32
NPBF16 = ml_dtypes.bfloat16


def kcol(kb):
    """kblock -> (partition base, col) in parity-packed kpT2 [128, 2048]."""
    return 64 * (kb % 2), PB * (kb // 2)


def build_nc():
    nc = bacc.Bacc(None)

    xq_d = nc.declare_dram_parameter("xq", [D, S // 2], BF16, isOutput=False)
    xk_d = nc.declare_dram_parameter("xk", [D, S], BF16, isOutput=False)
    xv_d = nc.declare_dram_parameter("xv", [D, S], BF16, isOutput=False)
    wq_d = nc.declare_dram_parameter("wq", [D, E], BF16, isOutput=False)
    wk_d = nc.declare_dram_parameter("wk", [D, E], BF16, isOutput=False)
    wv_d = nc.declare_dram_parameter("wv", [D, E], BF16, isOutput=False)
    cm_d = nc.declare_dram_parameter("cmask", [8, PB, CHW], BF16, isOutput=False)
    id_d = nc.declare_dram_parameter("ident", [PB, PB], F32, isOutput=False)
    out_d = nc.declare_dram_parameter("out", [S // 2, E], F32, isOutput=True)

    with tile.TileContext(nc) as tc:
        with tc.tile_pool(name="persist", bufs=1) as pp, \
             tc.tile_pool(name="st_ps", bufs=2, space="PSUM") as stp, \
             tc.tile_pool(name="zt_ps", bufs=1, space="PSUM") as ztp, \
             tc.tile_pool(name="zn_ps", bufs=1, space="PSUM") as znp, \
             tc.tile_pool(name="work", bufs=2 * LAG + 2) as wp, \
             tc.tile_pool(name="osb", bufs=3) as op:
            # ---- persistent SBUF tiles ----
            wq_sb = [pp.tile([PB, E], BF16, name=f"wq{i}", tag=f"wq{i}") for i in range(ND)]
            wk_sb = [pp.tile([PB, E], BF16, name=f"wk{i}", tag=f"wk{i}") for i in range(ND)]
            wv_sb = [pp.tile([PB, E], BF16, name=f"wv{i}", tag=f"wv{i}") for i in range(ND)]
            xq_sb = [pp.tile([PB, S // 2], BF16, name=f"xq{i}", tag=f"xq{i}") for i in range(ND)]
            xk_sb = [pp.tile([PB, S], BF16, name=f"xk{i}", tag=f"xk{i}") for i in range(ND)]
            xv_sb = [pp.tile([PB, S], BF16, name=f"xv{i}", tag=f"xv{i}") for i in range(ND)]
            mk_sb = [pp.tile([PB, CHW], BF16, name=f"mk{i}", tag=f"mk{i}") for i in range(8)]
            qpT2 = pp.tile([PB, S // 2], BF16, name="qpT2", tag="qpT2")
            kpT2 = pp.tile([PB, S // 2], BF16, name="kpT2", tag="kpT2")
            vpT = pp.tile([E, S], BF16, name="vpT", tag="vpT")
            vp = pp.tile([PB, NKB * (E + 1)], BF16, name="vp", tag="vp")
            idf_sb = pp.tile([PB, PB], F32, name="idf_sb", tag="idf_sb")
            idb_sb = pp.tile([PB, PB], BF16, name="idb_sb", tag="idb_sb")

            # ---- input DMAs (emission order = rough priority) ----
            for i in range(ND):
                nc.sync.dma_start(out=wq_sb[i][:], in_=wq_d[PB * i:PB * (i + 1), :])
                nc.sync.dma_start(out=wk_sb[i][:], in_=wk_d[PB * i:PB * (i + 1), :])
                nc.sync.dma_start(out=wv_sb[i][:], in_=wv_d[PB * i:PB * (i + 1), :])
            nc.sync.dma_start(out=idf_sb[:], in_=id_d[:])
            for i in range(ND):
                nc.sync.dma_start(out=xq_sb[i][:], in_=xq_d[PB * i:PB * (i + 1), :])
            for i in range(ND):
                nc.sync.dma_start(out=xk_sb[i][:], in_=xk_d[PB * i:PB * (i + 1), :])
            for i in range(ND):
                nc.sync.dma_start(out=xv_sb[i][:], in_=xv_d[PB * i:PB * (i + 1), :])
            for m in range(8):
                nc.sync.dma_start(out=mk_sb[m][:], in_=cm_d[m])
            nc.vector.tensor_copy(idb_sb[:], idf_sb[:])
            nc.vector.memset(vp[:], 1.0)  # pre-fill ones columns

            # ---- projections (psum slots shared with attention via tag) ----
            # Q projection -> qpT2 duplicated on both partition halves
            for c in range(NCH):
                qp_ps = stp.tile([E, CHW], F32, tag="st")
                for d in range(ND):
                    nc.tensor.matmul(
                        qp_ps[:], wq_sb[d][:], xq_sb[d][:, CHW * c:CHW * (c + 1)],
                        start=(d == 0), stop=(d == ND - 1))
                nc.vector.tensor_copy(qpT2[0:E, CHW * c:CHW * (c + 1)], qp_ps[:])
                nc.vector.tensor_copy(qpT2[E:2 * E, CHW * c:CHW * (c + 1)], qp_ps[:])
            # K projection -> kpT2 parity-packed
            for c in range(S // CHW):
                kp_ps = stp.tile([E, CHW], F32, tag="st")
                for d in range(ND):
                    nc.tensor.matmul(
                        kp_ps[:], wk_sb[d][:], xk_sb[d][:, CHW * c:CHW * (c + 1)],
                        start=(d == 0), stop=(d == ND - 1))
                for j in range(4):
                    kb = 4 * c + j
                    pb, col = kcol(kb)
                    nc.vector.tensor_copy(
                        kpT2[pb:pb + E, col:col + PB],
                        kp_ps[:, PB * j:PB * (j + 1)])
            # V projection e-major, then PE-transpose to k-major + ones col
            for c in range(S // CHW):
                vpp = stp.tile([E, CHW], F32, tag="st")
                for d in range(ND):
                    nc.tensor.matmul(
                        vpp[:], wv_sb[d][:], xv_sb[d][:, CHW * c:CHW * (c + 1)],
                        start=(d == 0), stop=(d == ND - 1))
                nc.vector.tensor_copy(vpT[:, CHW * c:CHW * (c + 1)], vpp[:])
            for s in range(NKB):
                vt_ps = stp.tile([PB, E], BF16, tag="st")
                nc.tensor.transpose(vt_ps[:], vpT[:, PB * s:PB * (s + 1)],
                                    idb_sb[0:E, 0:E])
                nc.vector.tensor_copy(vp[:, (E + 1) * s:(E + 1) * s + E], vt_ps[:])

            # ---- attention ----
            for c in range(NCH):
                nkb = 8 * c + 8          # padded kblock count for this chunk
                zt_ps = ztp.tile([E + 1, CHW], F32, tag="zt")
                groups = []
                k0 = 0
                while k0 < nkb:
                    g = min(GRP, nkb - k0)
                    groups.append(list(range(k0, k0 + g)))
                    k0 += g
                pend = []            # [(et_sb, kbs), ...] awaiting AV

                def drain_avs(p_et, p_kbs):
                    for ji, kb in enumerate(p_kbs):
                        nc.tensor.matmul(
                            zt_ps[:], vp[:, (E + 1) * kb:(E + 1) * (kb + 1)],
                            p_et[:, CHW * ji:CHW * (ji + 1)],
                            start=(p_kbs[0] == 0 and ji == 0),
                            stop=(p_kbs[-1] == nkb - 1 and ji == len(p_kbs) - 1),
                            skip_group_check=True)

                for gi, kbs in enumerate(groups):
                    gw = len(kbs) * CHW
                    st_ps = stp.tile([PB, GRP * CHW], F32, tag="st")
                    # score matmuls: row-tiled pair + (optional) single
                    pairs = [(0, 1)] if len(kbs) >= 2 else []
                    singles = list(range(2 if len(kbs) >= 2 else 0, len(kbs)))
                    for (a, b) in pairs:
                        for ji in (a, b):
                            kb = kbs[ji]
                            pb, col = kcol(kb)
                            nc.tensor.matmul(
                                st_ps[:, CHW * ji:CHW * (ji + 1)],
                                kpT2[pb:pb + E, col:col + PB],
                                qpT2[pb:pb + E, CHW * c:CHW * (c + 1)],
                                start=True, stop=True,
                                tile_position=(pb, 0))
                    for ji in singles:
                        kb = kbs[ji]
                        pb, col = kcol(kb)
                        nc.tensor.matmul(
                            st_ps[:, CHW * ji:CHW * (ji + 1)],
                            kpT2[pb:pb + E, col:col + PB],
                            qpT2[pb:pb + E, CHW * c:CHW * (c + 1)],
                            start=True, stop=True,
                            tile_position=(pb, 0))
                    # drain AVs lagging LAG groups behind
                    if len(pend) > LAG - 1:
                        drain_avs(*pend.pop(0))
                    et_sb = wp.tile([PB, GRP * CHW], BF16, tag="et")
                    nc.scalar.activation(
                        et_sb[:, :gw], st_ps[:, :gw],
                        mybir.ActivationFunctionType.Exp, scale=0.125)
                    for ji, kb in enumerate(kbs):
                        m = kb - 8 * c
                        if m >= 0:
                            nc.vector.tensor_mul(
                                et_sb[:, CHW * ji:CHW * (ji + 1)],
                                et_sb[:, CHW * ji:CHW * (ji + 1)],
                                mk_sb[m][:])
                    pend.append((et_sb, kbs))
                for p in pend:
                    drain_avs(*p)
                # ---- normalize via transpose (denominator = col E) ----
                zs_sb = wp.tile([E + 1, CHW], F32, tag="zs")
                nc.vector.tensor_copy(zs_sb[:], zt_ps[:])
                for j in range(4):
                    zn_ps = znp.tile([PB, E + 1], F32, tag="zn")
                    nc.tensor.transpose(zn_ps[:], zs_sb[:, PB * j:PB * (j + 1)],
                                        idf_sb[0:E + 1, 0:E + 1])
                    rc_sb = wp.tile([PB, 1], F32, tag="rc")
                    nc.vector.reciprocal(rc_sb[:], zn_ps[:, E:E + 1])
                    o_sb = op.tile([PB, E], F32, tag="osb")
                    nc.vector.tensor_scalar_mul(o_sb[:], zn_ps[:, 0:E], rc_sb[:])
                    q0 = PB * (4 * c + j)
                    nc.sync.dma_start(out=out_d[q0:q0 + PB, :], in_=o_sb[:])
    nc.finalize()
    return nc


def make_core_inputs(key_np, value_np, query_np, Wk, Wv, Wq):
    """Host-side sharding: returns in_maps list of 8 dicts."""
    bf = lambda a: np.ascontiguousarray(a).astype(NPBF16)
    in_maps = []
    for c in range(8):
        b, h = c // 2, c % 2
        qrows = np.concatenate(
            [np.arange(PB * (2 * j + h), PB * (2 * j + h) + PB) for j in range(NLQ)])
        # causal masks: mask m applies to kblock kb = 8c+m of every chunk;
        # section jj (q sub-block) has global q-block g = 8c+2jj+h,
        # class = m - 2jj - h: <0 keep, ==0 triangular, >0 zero.
        cmask = np.zeros((8, PB, CHW), dtype=np.float32)
        ki = np.arange(PB)[:, None]
        qi = np.arange(PB)[None, :]
        tri = (ki <= qi).astype(np.float32)
        for m in range(8):
            for jj in range(4):
                cls = m - 2 * jj - h
                blk = np.ones((PB, PB), np.float32) if cls < 0 else (
                    tri if cls == 0 else np.zeros((PB, PB), np.float32))
                cmask[m][:, PB * jj:PB * (jj + 1)] = blk
        in_maps.append({
            "xq": bf(query_np[b][qrows].T),
            "xk": bf(key_np[b].T),
            "xv": bf(value_np[b].T),
            "wq": bf(Wq), "wk": bf(Wk), "wv": bf(Wv),
            "cmask": bf(cmask),
            "ident": np.eye(PB, dtype=np.float32),
        })
    return in_maps


def assemble_output(results):
    """results: list of 8 dicts with 'out' [2048, 64] f32 -> Z [B,S,E]."""
    Z = np.zeros((B, S, E), dtype=np.float32)
    for c in range(8):
        b, h = c // 2, c % 2
        o = results[c]["out"]  # [2048, E] q-major
        for j in range(NLQ):
            g = 2 * j + h
            Z[b, PB * g:PB * (g + 1), :] = o[PB * j:PB * (j + 1), :]
    return Z


def kernel(key_inputs, value_inputs, query_inputs, Wk, Wv, Wq):
    from concourse.bass_utils import run_bass_kernel_spmd
    nc = build_nc()
    in_maps = make_core_inputs(np.asarray(key_inputs), np.asarray(value_inputs),
                               np.asarray(query_inputs), np.asarray(Wk),
                               np.asarray(Wv), np.asarray(Wq))
    res = run_bass_kernel_spmd(nc, in_maps, core_ids=list(range(8)))
    return assemble_output(res.results)
